# revision 31
# baseline (speedup 1.0000x reference)
"""Trainium2 Bass kernel for dual-branch (low-rank + full-rank) self-attention.

Math (per batch b, head h):
  q = x @ Wq_cat[h].T   (N, 224)   224 = 64 (lr) + 160 (full)
  scoresT[m, n] = sum_d K[m, d] Q[n, d]           (keys m on partitions)
  expT = exp(SCALE * scoresT)                     (no max subtraction; f32 psum)
  xav[d, n] = sum_m Vaug[m, d] expT[m, n]         Vaug has a ones column ->
                                                  row 224 of xav = softmax denom
  xnorm = xav * (1/denom)  (PE K-broadcast of the reciprocal row)
  outT[c, n] = sum_{h,d} Wout_aug[h*225+d, c] * xnorm[h][d, n]
               (ones-row of xnorm x bias-row of Wout_aug adds the bias)

Sharding: data-parallel, 2 batches per core across 8 cores. No collectives.
All matmuls bf16 with f32 PSUM accumulation; softmax + normalization f32.
"""

import os
import sys

sys.path.insert(0, "/opt/trn_rl_repo")

import numpy as np
import ml_dtypes

import concourse.bass as bass
import concourse.mybir as mybir
import concourse.tile as tile
from concourse import bacc
from concourse.bass_utils import run_bass_kernel_spmd

# problem constants (hardcoded per spec)
B, N, C = 16, 1024, 1280
HEADS = 8
RANK = 64
DIM_HEAD = 160
DH = RANK + DIM_HEAD          # 224 concat head dim
SCALE = DIM_HEAD ** (-0.5)
NCORES = 8
BL = B // NCORES              # batches per core = 2
CK = C // 128                 # 10 contraction chunks over C
GROUPS = 2                    # head groups per core pass
HG = HEADS // GROUPS          # 4 heads per group
MC = N // 128                 # 8 key chunks
NT = N // 512                 # 2 query-column tiles

BF16 = mybir.dt.bfloat16
F32 = mybir.dt.float32


def build_bass():
    nc = bacc.Bacc("TRN2", target_bir_lowering=False, debug=False,
                   num_devices=NCORES)

    def din(name, shape, dt=BF16):
        return nc.dram_tensor(name, shape, dt, kind="ExternalInput").ap()

    xt_d = din("xt", [128, CK, BL * N])                 # x transposed, c-major
    wq_lo_d = din("wq_lo", [GROUPS, 128, CK, HG * 128])
    wq_hi_d = din("wq_hi", [GROUPS, 128, CK, HG * 96])
    wk_lo_d = din("wk_lo", [GROUPS, 128, CK, HG * 128])
    wk_hi_d = din("wk_hi", [GROUPS, 128, CK, HG * 96])
    wv_lo_d = din("wv_lo", [GROUPS, 128, CK, HG * 128])
    wv_hi_d = din("wv_hi", [GROUPS, 128, CK, HG * 97])  # 97th col zero (ones col)
    wo_lo_d = din("wo_lo", [HEADS, 128, C])
    wo_hi_d = din("wo_hi", [HEADS, 128, C])             # rows 96..127 zero; row 96 of h=0 = bias
    ind_d = din("ind", [128, 8 * 128])                  # row-broadcast indicator columns
    out_d = nc.dram_tensor("out", [BL, CK, 128, N], F32, kind="ExternalOutput").ap()

    with tile.TileContext(nc) as tc:
        with (
            tc.tile_pool(name="xtp", bufs=1) as xtp,
            tc.tile_pool(name="wp", bufs=3) as wp,
            tc.tile_pool(name="wop", bufs=16) as wop,
            tc.tile_pool(name="qkvp", bufs=1) as qkvp,
            tc.tile_pool(name="xavp", bufs=1) as xavp,
            tc.tile_pool(name="expp", bufs=2) as expp,
            tc.tile_pool(name="cstp", bufs=1) as cstp,
            tc.tile_pool(name="outp", bufs=2) as outp,
            tc.tile_pool(name="xunp", bufs=2) as xunp,
            tc.tile_pool(name="drp", bufs=2, space="DRAM") as drp,
            tc.tile_pool(name="psp", bufs=5, space="PSUM") as psp,
            tc.tile_pool(name="psavp", bufs=3, space="PSUM") as psavp,
        ):
            def ps_tile():
                # general-purpose matmul accumulators (scores/proj/out-proj)
                return psp.tile([128, 512], F32, tag="mm", name="ps")

            def ps_av_tile():
                # AV accumulators + broadcast — separate slots so the
                # normalize chain never blocks the next head's scores
                return psavp.tile([128, 512], F32, tag="av", name="ps_av")

            # constants: indicator columns for row-broadcast matmuls
            # ind[:, r*128:(r+1)*128] has ones in row r only, so
            # matmul(out, ind[:, r-slice], recip_all) broadcasts row r

            for b in range(BL):
                xt = xtp.tile([128, CK, N], BF16, tag="xt")
                for co in range(CK):  # per-chunk loads: compute starts sooner
                    nc.sync.dma_start(xt[:, co, :],
                                      xt_d[:, co, b * N:(b + 1) * N])

                xav_lo = xavp.tile([128, HEADS, N], BF16, tag="xav_lo")
                xav_hi = xavp.tile([128, HEADS, N], BF16, tag="xav_hi")
                nc.gpsimd.memset(xav_hi[96:128, :, :], 0.0)

                for g in range(GROUPS):
                    # ---- stream this group's projection weights ----
                    def load_slab(dram, width):
                        t = wp.tile([128, CK, width], BF16, tag="wslab",
                                    name="wslab")
                        for co in range(CK):  # per-chunk: spread across queues
                            nc.sync.dma_start(t[:, co, :], dram[g, :, co, :])
                        return t
                    w_qlo = load_slab(wq_lo_d, HG * 128)
                    w_qhi = load_slab(wq_hi_d, HG * 96)
                    w_klo = load_slab(wk_lo_d, HG * 128)
                    w_khi = load_slab(wk_hi_d, HG * 96)
                    w_vlo = load_slab(wv_lo_d, HG * 128)
                    w_vhi = load_slab(wv_hi_d, HG * 97)

                    # ---- projections: qT/kT (head-dim on partitions) ----
                    qt_lo = qkvp.tile([128, HG, N], BF16, tag="qt_lo")
                    qt_hi = qkvp.tile([128, HG, N], BF16, tag="qt_hi")
                    kt_lo = qkvp.tile([128, HG, N], BF16, tag="kt_lo")
                    kt_hi = qkvp.tile([128, HG, N], BF16, tag="kt_hi")
                    nc.gpsimd.memset(qt_hi[96:128, :, :], 0.0)
                    nc.gpsimd.memset(kt_hi[96:128, :, :], 0.0)

                    # hi chunks of 4 heads (4x96=384 rows) pack into 3 full
                    # M=128 matmuls; fragments unmixed by the psum->sbuf
                    # copies (pieces respect the 32-aligned partition-window
                    # rule: base 0 may span freely, other bases max 32 rows)
                    HI_FRAGS = {  # ch -> [(hl, src_base, dst_base, rows)]
                        0: [(0, 0, 0, 96), (1, 96, 0, 32)],
                        1: [(1, 0, 32, 32), (1, 32, 64, 32),
                            (2, 64, 0, 32), (2, 96, 32, 32)],
                        2: [(2, 0, 64, 32), (3, 32, 0, 32),
                            (3, 64, 32, 32), (3, 96, 64, 32)],
                    }
                    for (wlo, whi, tlo, thi) in (
                        (w_qlo, w_qhi, qt_lo, qt_hi),
                        (w_klo, w_khi, kt_lo, kt_hi),
                    ):
                        for hl in range(HG):
                            for nt in range(NT):
                                ps = ps_tile()
                                for co in range(CK):
                                    nc.tensor.matmul(
                                        ps[:],
                                        wlo[:, co, hl * 128:(hl + 1) * 128],
                                        xt[:, co, nt * 512:(nt + 1) * 512],
                                        start=(co == 0), stop=(co == CK - 1))
                                nc.vector.tensor_copy(
                                    tlo[:, hl, nt * 512:(nt + 1) * 512], ps[:])
                        for ch in range(3):
                            for nt in range(NT):
                                ps = ps_tile()
                                for co in range(CK):
                                    nc.tensor.matmul(
                                        ps[:],
                                        whi[:, co, ch * 128:(ch + 1) * 128],
                                        xt[:, co, nt * 512:(nt + 1) * 512],
                                        start=(co == 0), stop=(co == CK - 1))
                                for (hl, sb, db, rows) in HI_FRAGS[ch]:
                                    nc.vector.tensor_copy(
                                        thi[db:db + rows, hl,
                                            nt * 512:(nt + 1) * 512],
                                        ps[sb:sb + rows, :])

                    # ---- V projection: natural layout (keys on partitions) ----
                    v_lo = qkvp.tile([128, MC, HG * 128], BF16, tag="v_lo")
                    v_hi = qkvp.tile([128, MC, HG * 97], BF16, tag="v_hi")
                    for mc in range(MC):
                        ps_l = ps_tile()
                        ps_h = ps_tile()
                        for co in range(CK):
                            nc.tensor.matmul(
                                ps_l[:],
                                xt[:, co, mc * 128:(mc + 1) * 128],
                                w_vlo[:, co, :],
                                start=(co == 0), stop=(co == CK - 1))
                        for co in range(CK):
                            nc.tensor.matmul(
                                ps_h[:, 0:HG * 97],
                                xt[:, co, mc * 128:(mc + 1) * 128],
                                w_vhi[:, co, :],
                                start=(co == 0), stop=(co == CK - 1))
                        nc.vector.tensor_copy(v_lo[:, mc, :], ps_l[:])
                        nc.vector.tensor_copy(v_hi[:, mc, :], ps_h[:, 0:HG * 97])
                    # ones column per head (softmax denominator row source)
                    for hl in range(HG):
                        nc.gpsimd.memset(v_hi[:, :, hl * 97 + 96], 1.0)

                    # ---- attention per head: unnormalized AV into xav ----
                    for hl in range(HG):
                        h_abs = g * HG + hl
                        for nt in range(NT):
                            expt = expp.tile([128, MC * 512], BF16, tag="expt")
                            for mc in range(MC):
                                ps_sc = ps_tile()
                                nc.tensor.matmul(
                                    ps_sc[:],
                                    kt_lo[:, hl, mc * 128:(mc + 1) * 128],
                                    qt_lo[:, hl, nt * 512:(nt + 1) * 512],
                                    start=True, stop=False)
                                nc.tensor.matmul(
                                    ps_sc[:],
                                    kt_hi[:, hl, mc * 128:(mc + 1) * 128],
                                    qt_hi[:, hl, nt * 512:(nt + 1) * 512],
                                    start=False, stop=True)
                                nc.scalar.activation(
                                    expt[:, mc * 512:(mc + 1) * 512],
                                    ps_sc[:],
                                    mybir.ActivationFunctionType.Exp,
                                    scale=SCALE)
                            ps_alo = ps_av_tile()
                            ps_ahi = ps_av_tile()
                            for mc in range(MC):
                                nc.tensor.matmul(
                                    ps_alo[:],
                                    v_lo[:, mc, hl * 128:(hl + 1) * 128],
                                    expt[:, mc * 512:(mc + 1) * 512],
                                    start=(mc == 0), stop=(mc == MC - 1))
                            for mc in range(MC):
                                nc.tensor.matmul(
                                    ps_ahi[0:97, :],
                                    v_hi[:, mc, hl * 97:(hl + 1) * 97],
                                    expt[:, mc * 512:(mc + 1) * 512],
                                    start=(mc == 0), stop=(mc == MC - 1))
                            # drain PSUM immediately (unnormalized)
                            nc.vector.tensor_copy(
                                xav_lo[:, h_abs, nt * 512:(nt + 1) * 512],
                                ps_alo[:])
                            nc.vector.tensor_copy(
                                xav_hi[0:97, h_abs, nt * 512:(nt + 1) * 512],
                                ps_ahi[0:97, :])

                    # gather this group's 8 denominator rows onto 8 partitions
                    # (DMA via DRAM bounce: compute engines can't write
                    # partition offsets 1..7, and SBUF APs can't move free
                    # elements across partitions)
                    s_scr = drp.tile([1, HG, N], BF16, tag="s_scr")
                    nc.sync.dma_start(
                        s_scr[:], xav_hi[96:97, g * HG:(g + 1) * HG, :])
                    s_all = xunp.tile([8, 512], BF16, tag="s_all")
                    nc.sync.dma_start(
                        s_all[:],
                        s_scr[:].rearrange("o h (t i) -> (o h t) i", t=NT))

                    # ---- batched softmax denominators: one 8-lane recip,
                    # then partition-broadcast via stride-0 DRAM-read DMAs ----
                    recip_all = xunp.tile([8, 512], BF16, tag="recip_all")
                    with nc.allow_low_precision(reason="softmax denom recip bf16"):
                        nc.vector.reciprocal(recip_all[:], s_all[:])
                    r_scr = drp.tile([8, 512], BF16, tag="r_scr")
                    nc.sync.dma_start(r_scr[:], recip_all[:])
                    for hl in range(HG):
                        h_abs = g * HG + hl
                        for nt in range(NT):
                            sel = hl * NT + nt
                            bc_sb = outp.tile([128, 512], BF16, tag="bc_sb")
                            nc.sync.dma_start(
                                bc_sb[:],
                                r_scr[sel:sel + 1, :].to_broadcast([128, 512]))
                            nc.gpsimd.tensor_tensor(
                                xav_lo[:, h_abs, nt * 512:(nt + 1) * 512],
                                xav_lo[:, h_abs, nt * 512:(nt + 1) * 512],
                                bc_sb[:], mybir.AluOpType.mult)
                            nc.gpsimd.tensor_tensor(
                                xav_hi[0:97, h_abs, nt * 512:(nt + 1) * 512],
                                xav_hi[0:97, h_abs, nt * 512:(nt + 1) * 512],
                                bc_sb[0:97, :], mybir.AluOpType.mult)

                # ---- output projection (c on partitions; host untransposes) ----
                wo_lo_t = []
                wo_hi_t = []
                for h in range(HEADS):
                    wl = wop.tile([128, C], BF16, tag="wo")
                    nc.sync.dma_start(wl[:], wo_lo_d[h])
                    wh = wop.tile([128, C], BF16, tag="wo")
                    nc.sync.dma_start(wh[:], wo_hi_d[h])
                    wo_lo_t.append(wl)
                    wo_hi_t.append(wh)
                for ct in range(CK):
                    for nt in range(NT):
                        ps_o = ps_tile()
                        for h in range(HEADS):
                            nc.tensor.matmul(
                                ps_o[:],
                                wo_lo_t[h][:, ct * 128:(ct + 1) * 128],
                                xav_lo[:, h, nt * 512:(nt + 1) * 512],
                                start=(h == 0), stop=False)
                            nc.tensor.matmul(
                                ps_o[:],
                                wo_hi_t[h][:, ct * 128:(ct + 1) * 128],
                                xav_hi[:, h, nt * 512:(nt + 1) * 512],
                                start=False, stop=(h == HEADS - 1))
                        ot = outp.tile([128, 512], F32, tag="ot")
                        nc.vector.tensor_copy(ot[:], ps_o[:])
                        nc.sync.dma_start(
                            out_d[b, ct, :, nt * 512:(nt + 1) * 512], ot[:])

    nc.compile()
    return nc


def _prep_weights(Wq_lr, Wk_lr, Wv_lr, Wout_lr, Wq_full, Wk_full, Wv_full,
                  Wout_full, b_out_full):
    """Host-side weight concat/transpose into device layouts (bf16)."""
    bf16 = ml_dtypes.bfloat16

    def cat_heads(W_lr, W_full):
        # -> (H, 224, C)
        lr = W_lr.reshape(HEADS, RANK, C)
        fl = W_full.reshape(HEADS, DIM_HEAD, C)
        return np.concatenate([lr, fl], axis=1)

    def slab_lo(Wcat):
        # (H,224,C) -> per group [G, 128(p), CK, HG*128] with layout
        # [g][p, co, hl*128+j] = Wcat[g*HG+hl, j, co*128+p]
        A = Wcat[:, :128, :].reshape(GROUPS, HG, 128, CK, 128)
        return np.ascontiguousarray(A.transpose(0, 4, 3, 1, 2)
                                    .reshape(GROUPS, 128, CK, HG * 128)
                                    ).astype(bf16)

    def slab_hi(Wcat, width, pad_to=None):
        A = Wcat[:, 128:224, :].reshape(GROUPS, HG, 96, CK, 128)
        A = A.transpose(0, 4, 3, 1, 2)  # (G, p, co, hl, 96)
        if pad_to is not None:
            pad = np.zeros(A.shape[:-1] + (pad_to - 96,), A.dtype)
            A = np.concatenate([A, pad], axis=-1)
            width = pad_to
        return np.ascontiguousarray(
            A.reshape(GROUPS, 128, CK, HG * width)).astype(bf16)

    Wq_cat = cat_heads(Wq_lr, Wq_full)
    Wk_cat = cat_heads(Wk_lr, Wk_full)
    Wv_cat = cat_heads(Wv_lr, Wv_full)

    # output projection: Wo_cat (H, 224, C) with Wo_cat[h, d, c] = Wout_cat[c, h*224+d]
    Wo_lr = Wout_lr.reshape(C, HEADS, RANK)
    Wo_fl = Wout_full.reshape(C, HEADS, DIM_HEAD)
    Wo_cat = np.concatenate([Wo_lr, Wo_fl], axis=2).transpose(1, 2, 0)  # (H,224,C)
    wo_lo = np.ascontiguousarray(Wo_cat[:, :128, :]).astype(bf16)
    wo_hi = np.zeros((HEADS, 128, C), np.float32)
    wo_hi[:, :96, :] = Wo_cat[:, 128:, :]
    wo_hi[0, 96, :] = b_out_full  # ones-row of head 0 carries the bias
    wo_hi = wo_hi.astype(bf16)

    ind = np.zeros((128, 8 * 128), np.float32)
    for r in range(8):
        ind[r, r * 128:(r + 1) * 128] = 1.0

    return {
        "wq_lo": slab_lo(Wq_cat), "wq_hi": slab_hi(Wq_cat, 96),
        "wk_lo": slab_lo(Wk_cat), "wk_hi": slab_hi(Wk_cat, 96),
        "wv_lo": slab_lo(Wv_cat), "wv_hi": slab_hi(Wv_cat, 96, pad_to=97),
        "wo_lo": wo_lo, "wo_hi": wo_hi, "ind": ind.astype(bf16),
    }


def _prep_xt(hs_core):
    # (BL, N, C) f32 -> [128, CK, BL*N] bf16, xt[p, co, b*N+n] = x[b, n, co*128+p]
    X = hs_core.reshape(BL * N, CK, 128).transpose(2, 1, 0)
    return np.ascontiguousarray(X).astype(ml_dtypes.bfloat16)


_NC_CACHE = {}


def get_nc():
    if "nc" not in _NC_CACHE:
        _NC_CACHE["nc"] = build_bass()
    return _NC_CACHE["nc"]


def kernel(hidden_states, Wq_lr, Wk_lr, Wv_lr, Wout_lr,
           Wq_full, Wk_full, Wv_full, Wout_full, b_out_full):
    hidden_states = np.asarray(hidden_states, np.float32)
    weights = _prep_weights(
        np.asarray(Wq_lr, np.float32), np.asarray(Wk_lr, np.float32),
        np.asarray(Wv_lr, np.float32), np.asarray(Wout_lr, np.float32),
        np.asarray(Wq_full, np.float32), np.asarray(Wk_full, np.float32),
        np.asarray(Wv_full, np.float32), np.asarray(Wout_full, np.float32),
        np.asarray(b_out_full, np.float32))

    in_maps = []
    for c in range(NCORES):
        m = dict(weights)
        m["xt"] = _prep_xt(hidden_states[c * BL:(c + 1) * BL])
        in_maps.append(m)

    nc = get_nc()
    results = run_bass_kernel_spmd(nc, in_maps, core_ids=list(range(NCORES))).results

    out = np.empty((B, N, C), np.float32)
    for c in range(NCORES):
        o = results[c]["out"]  # (BL, CK, 128, N)
        out[c * BL:(c + 1) * BL] = (
            o.transpose(0, 3, 1, 2).reshape(BL, N, C))
    return out


if __name__ == "__main__":
    nc = get_nc()
    print("built + compiled OK")


# revision 32
# speedup vs baseline: 1.0308x; 1.0308x over previous
"""Trainium2 Bass kernel for dual-branch (low-rank + full-rank) self-attention.

Math (per batch b, head h):
  q = x @ Wq_cat[h].T   (N, 224)   224 = 64 (lr) + 160 (full)
  scoresT[m, n] = sum_d K[m, d] Q[n, d]           (keys m on partitions)
  expT = exp(SCALE * scoresT)                     (no max subtraction; f32 psum)
  xav[d, n] = sum_m Vaug[m, d] expT[m, n]         Vaug has a ones column ->
                                                  row 224 of xav = softmax denom
  xnorm = xav * (1/denom)  (PE K-broadcast of the reciprocal row)
  outT[c, n] = sum_{h,d} Wout_aug[h*225+d, c] * xnorm[h][d, n]
               (ones-row of xnorm x bias-row of Wout_aug adds the bias)

Sharding: data-parallel, 2 batches per core across 8 cores. No collectives.
All matmuls bf16 with f32 PSUM accumulation; softmax + normalization f32.
"""

import os
import sys

sys.path.insert(0, "/opt/trn_rl_repo")

import numpy as np
import ml_dtypes

import concourse.bass as bass
import concourse.mybir as mybir
import concourse.tile as tile
from concourse import bacc
from concourse.bass_utils import run_bass_kernel_spmd

# problem constants (hardcoded per spec)
B, N, C = 16, 1024, 1280
HEADS = 8
RANK = 64
DIM_HEAD = 160
DH = RANK + DIM_HEAD          # 224 concat head dim
SCALE = DIM_HEAD ** (-0.5)
NCORES = 8
BL = B // NCORES              # batches per core = 2
CK = C // 128                 # 10 contraction chunks over C
GROUPS = 2                    # head groups per core pass
HG = HEADS // GROUPS          # 4 heads per group
MC = N // 128                 # 8 key chunks
NT = N // 512                 # 2 query-column tiles

BF16 = mybir.dt.bfloat16
F32 = mybir.dt.float32


def build_bass():
    nc = bacc.Bacc("TRN2", target_bir_lowering=False, debug=False,
                   num_devices=NCORES)

    def din(name, shape, dt=BF16):
        return nc.dram_tensor(name, shape, dt, kind="ExternalInput").ap()

    xt_d = din("xt", [128, CK, BL * N])                 # x transposed, c-major
    wq_lo_d = din("wq_lo", [GROUPS, 128, CK, HG * 128])
    wq_hi_d = din("wq_hi", [GROUPS, 128, CK, HG * 96])
    wk_lo_d = din("wk_lo", [GROUPS, 128, CK, HG * 128])
    wk_hi_d = din("wk_hi", [GROUPS, 128, CK, HG * 96])
    wv_lo_d = din("wv_lo", [GROUPS, 128, CK, HG * 128])
    wv_hi_d = din("wv_hi", [GROUPS, 128, CK, HG * 97])  # 97th col zero (ones col)
    wo_lo_d = din("wo_lo", [HEADS, 128, C])
    wo_hi_d = din("wo_hi", [HEADS, 128, C])             # rows 96..127 zero; row 96 of h=0 = bias
    ind_d = din("ind", [128, 8 * 128])                  # row-broadcast indicator columns
    out_d = nc.dram_tensor("out", [BL, CK, 128, N], F32, kind="ExternalOutput").ap()

    with tile.TileContext(nc) as tc:
        with (
            tc.tile_pool(name="xtp", bufs=1) as xtp,
            tc.tile_pool(name="wp", bufs=3) as wp,
            tc.tile_pool(name="wop", bufs=16) as wop,
            tc.tile_pool(name="qkvp", bufs=1) as qkvp,
            tc.tile_pool(name="xavp", bufs=1) as xavp,
            tc.tile_pool(name="expp", bufs=2) as expp,
            tc.tile_pool(name="cstp", bufs=1) as cstp,
            tc.tile_pool(name="outp", bufs=2) as outp,
            tc.tile_pool(name="xunp", bufs=2) as xunp,
            tc.tile_pool(name="drp", bufs=2, space="DRAM") as drp,
            tc.tile_pool(name="psp", bufs=5, space="PSUM") as psp,
            tc.tile_pool(name="psavp", bufs=3, space="PSUM") as psavp,
        ):
            def ps_tile():
                # general-purpose matmul accumulators (scores/proj/out-proj)
                return psp.tile([128, 512], F32, tag="mm", name="ps")

            def ps_av_tile():
                # AV accumulators + broadcast — separate slots so the
                # normalize chain never blocks the next head's scores
                return psavp.tile([128, 512], F32, tag="av", name="ps_av")

            # constants: indicator columns for row-broadcast matmuls
            # ind[:, r*128:(r+1)*128] has ones in row r only, so
            # matmul(out, ind[:, r-slice], recip_all) broadcasts row r

            pending_out = []

            def emit_out_proj():
                # output projection (c on partitions; host untransposes)
                ob, oxav_lo, oxav_hi = pending_out.pop(0)
                wo_lo_t = []
                wo_hi_t = []
                for h in range(HEADS):
                    wl = wop.tile([128, C], BF16, tag="wo", name="wo")
                    nc.sync.dma_start(wl[:], wo_lo_d[h])
                    wh = wop.tile([128, C], BF16, tag="wo", name="wo")
                    nc.sync.dma_start(wh[:], wo_hi_d[h])
                    wo_lo_t.append(wl)
                    wo_hi_t.append(wh)
                for ct in range(CK):
                    for nt in range(NT):
                        ps_o = ps_tile()
                        for h in range(HEADS):
                            nc.tensor.matmul(
                                ps_o[:],
                                wo_lo_t[h][:, ct * 128:(ct + 1) * 128],
                                oxav_lo[:, h, nt * 512:(nt + 1) * 512],
                                start=(h == 0), stop=False)
                            nc.tensor.matmul(
                                ps_o[:],
                                wo_hi_t[h][:, ct * 128:(ct + 1) * 128],
                                oxav_hi[:, h, nt * 512:(nt + 1) * 512],
                                start=False, stop=(h == HEADS - 1))
                        ot = outp.tile([128, 512], F32, tag="ot", name="ot")
                        nc.vector.tensor_copy(ot[:], ps_o[:])
                        nc.sync.dma_start(
                            out_d[ob, ct, :, nt * 512:(nt + 1) * 512], ot[:])

            for b in range(BL):
                xt = xtp.tile([128, CK, N], BF16, tag="xt")
                for co in range(CK):  # per-chunk loads: compute starts sooner
                    nc.sync.dma_start(xt[:, co, :],
                                      xt_d[:, co, b * N:(b + 1) * N])

                xav_lo = xavp.tile([128, HEADS, N], BF16, tag="xav_lo")
                xav_hi = xavp.tile([128, HEADS, N], BF16, tag="xav_hi")
                nc.gpsimd.memset(xav_hi[96:128, :, :], 0.0)

                for g in range(GROUPS):
                    # ---- stream this group's projection weights ----
                    def load_slab(dram, width):
                        t = wp.tile([128, CK, width], BF16, tag="wslab",
                                    name="wslab")
                        for co in range(CK):  # per-chunk: spread across queues
                            nc.sync.dma_start(t[:, co, :], dram[g, :, co, :])
                        return t
                    w_qlo = load_slab(wq_lo_d, HG * 128)
                    w_qhi = load_slab(wq_hi_d, HG * 96)
                    w_klo = load_slab(wk_lo_d, HG * 128)
                    w_khi = load_slab(wk_hi_d, HG * 96)
                    w_vlo = load_slab(wv_lo_d, HG * 128)
                    w_vhi = load_slab(wv_hi_d, HG * 97)

                    # ---- projections: qT/kT (head-dim on partitions) ----
                    qt_lo = qkvp.tile([128, HG, N], BF16, tag="qt_lo")
                    qt_hi = qkvp.tile([128, HG, N], BF16, tag="qt_hi")
                    kt_lo = qkvp.tile([128, HG, N], BF16, tag="kt_lo")
                    kt_hi = qkvp.tile([128, HG, N], BF16, tag="kt_hi")
                    nc.gpsimd.memset(qt_hi[96:128, :, :], 0.0)
                    nc.gpsimd.memset(kt_hi[96:128, :, :], 0.0)

                    # hi chunks of 4 heads (4x96=384 rows) pack into 3 full
                    # M=128 matmuls; fragments unmixed by the psum->sbuf
                    # copies (pieces respect the 32-aligned partition-window
                    # rule: base 0 may span freely, other bases max 32 rows)
                    HI_FRAGS = {  # ch -> [(hl, src_base, dst_base, rows)]
                        0: [(0, 0, 0, 96), (1, 96, 0, 32)],
                        1: [(1, 0, 32, 32), (1, 32, 64, 32),
                            (2, 64, 0, 32), (2, 96, 32, 32)],
                        2: [(2, 0, 64, 32), (3, 32, 0, 32),
                            (3, 64, 32, 32), (3, 96, 64, 32)],
                    }
                    for (wlo, whi, tlo, thi) in (
                        (w_qlo, w_qhi, qt_lo, qt_hi),
                        (w_klo, w_khi, kt_lo, kt_hi),
                    ):
                        for hl in range(HG):
                            for nt in range(NT):
                                ps = ps_tile()
                                for co in range(CK):
                                    nc.tensor.matmul(
                                        ps[:],
                                        wlo[:, co, hl * 128:(hl + 1) * 128],
                                        xt[:, co, nt * 512:(nt + 1) * 512],
                                        start=(co == 0), stop=(co == CK - 1))
                                nc.vector.tensor_copy(
                                    tlo[:, hl, nt * 512:(nt + 1) * 512], ps[:])
                        for ch in range(3):
                            for nt in range(NT):
                                ps = ps_tile()
                                for co in range(CK):
                                    nc.tensor.matmul(
                                        ps[:],
                                        whi[:, co, ch * 128:(ch + 1) * 128],
                                        xt[:, co, nt * 512:(nt + 1) * 512],
                                        start=(co == 0), stop=(co == CK - 1))
                                for (hl, sb, db, rows) in HI_FRAGS[ch]:
                                    nc.vector.tensor_copy(
                                        thi[db:db + rows, hl,
                                            nt * 512:(nt + 1) * 512],
                                        ps[sb:sb + rows, :])

                    # ---- V projection: natural layout (keys on partitions) ----
                    v_lo = qkvp.tile([128, MC, HG * 128], BF16, tag="v_lo")
                    v_hi = qkvp.tile([128, MC, HG * 97], BF16, tag="v_hi")
                    for mc in range(MC):
                        ps_l = ps_tile()
                        ps_h = ps_tile()
                        for co in range(CK):
                            nc.tensor.matmul(
                                ps_l[:],
                                xt[:, co, mc * 128:(mc + 1) * 128],
                                w_vlo[:, co, :],
                                start=(co == 0), stop=(co == CK - 1))
                        for co in range(CK):
                            nc.tensor.matmul(
                                ps_h[:, 0:HG * 97],
                                xt[:, co, mc * 128:(mc + 1) * 128],
                                w_vhi[:, co, :],
                                start=(co == 0), stop=(co == CK - 1))
                        nc.vector.tensor_copy(v_lo[:, mc, :], ps_l[:])
                        nc.vector.tensor_copy(v_hi[:, mc, :], ps_h[:, 0:HG * 97])
                    # ones column per head (softmax denominator row source)
                    for hl in range(HG):
                        nc.gpsimd.memset(v_hi[:, :, hl * 97 + 96], 1.0)

                    # emit the previous batch's output projection here so
                    # the PE chews it while this batch's attention deps settle
                    if g == 0 and pending_out:
                        emit_out_proj()

                    # ---- attention per head: unnormalized AV into xav ----
                    for hl in range(HG):
                        h_abs = g * HG + hl
                        for nt in range(NT):
                            expt = expp.tile([128, MC * 512], BF16, tag="expt")
                            for mc in range(MC):
                                ps_sc = ps_tile()
                                nc.tensor.matmul(
                                    ps_sc[:],
                                    kt_lo[:, hl, mc * 128:(mc + 1) * 128],
                                    qt_lo[:, hl, nt * 512:(nt + 1) * 512],
                                    start=True, stop=False)
                                nc.tensor.matmul(
                                    ps_sc[:],
                                    kt_hi[:, hl, mc * 128:(mc + 1) * 128],
                                    qt_hi[:, hl, nt * 512:(nt + 1) * 512],
                                    start=False, stop=True)
                                nc.scalar.activation(
                                    expt[:, mc * 512:(mc + 1) * 512],
                                    ps_sc[:],
                                    mybir.ActivationFunctionType.Exp,
                                    scale=SCALE)
                            ps_alo = ps_av_tile()
                            ps_ahi = ps_av_tile()
                            for mc in range(MC):
                                nc.tensor.matmul(
                                    ps_alo[:],
                                    v_lo[:, mc, hl * 128:(hl + 1) * 128],
                                    expt[:, mc * 512:(mc + 1) * 512],
                                    start=(mc == 0), stop=(mc == MC - 1))
                            for mc in range(MC):
                                nc.tensor.matmul(
                                    ps_ahi[0:97, :],
                                    v_hi[:, mc, hl * 97:(hl + 1) * 97],
                                    expt[:, mc * 512:(mc + 1) * 512],
                                    start=(mc == 0), stop=(mc == MC - 1))
                            # drain PSUM immediately (unnormalized)
                            nc.vector.tensor_copy(
                                xav_lo[:, h_abs, nt * 512:(nt + 1) * 512],
                                ps_alo[:])
                            nc.vector.tensor_copy(
                                xav_hi[0:97, h_abs, nt * 512:(nt + 1) * 512],
                                ps_ahi[0:97, :])

                    # gather this group's 8 denominator rows onto 8 partitions
                    # (DMA via DRAM bounce: compute engines can't write
                    # partition offsets 1..7, and SBUF APs can't move free
                    # elements across partitions)
                    s_scr = drp.tile([1, HG, N], BF16, tag="s_scr")
                    nc.sync.dma_start(
                        s_scr[:], xav_hi[96:97, g * HG:(g + 1) * HG, :])
                    s_all = xunp.tile([8, 512], BF16, tag="s_all")
                    nc.sync.dma_start(
                        s_all[:],
                        s_scr[:].rearrange("o h (t i) -> (o h t) i", t=NT))

                    # ---- batched softmax denominators: one 8-lane recip,
                    # then partition-broadcast via stride-0 DRAM-read DMAs ----
                    recip_all = xunp.tile([8, 512], BF16, tag="recip_all")
                    with nc.allow_low_precision(reason="softmax denom recip bf16"):
                        nc.vector.reciprocal(recip_all[:], s_all[:])
                    r_scr = drp.tile([8, 512], BF16, tag="r_scr")
                    nc.sync.dma_start(r_scr[:], recip_all[:])
                    for hl in range(HG):
                        h_abs = g * HG + hl
                        for nt in range(NT):
                            sel = hl * NT + nt
                            bc_sb = outp.tile([128, 512], BF16, tag="bc_sb")
                            nc.sync.dma_start(
                                bc_sb[:],
                                r_scr[sel:sel + 1, :].to_broadcast([128, 512]))
                            nc.vector.tensor_tensor(
                                xav_lo[:, h_abs, nt * 512:(nt + 1) * 512],
                                xav_lo[:, h_abs, nt * 512:(nt + 1) * 512],
                                bc_sb[:], mybir.AluOpType.mult)
                            nc.vector.tensor_tensor(
                                xav_hi[0:97, h_abs, nt * 512:(nt + 1) * 512],
                                xav_hi[0:97, h_abs, nt * 512:(nt + 1) * 512],
                                bc_sb[0:97, :], mybir.AluOpType.mult)

                pending_out.append((b, xav_lo, xav_hi))

            while pending_out:
                emit_out_proj()

    nc.compile()
    return nc


def _prep_weights(Wq_lr, Wk_lr, Wv_lr, Wout_lr, Wq_full, Wk_full, Wv_full,
                  Wout_full, b_out_full):
    """Host-side weight concat/transpose into device layouts (bf16)."""
    bf16 = ml_dtypes.bfloat16

    def cat_heads(W_lr, W_full):
        # -> (H, 224, C)
        lr = W_lr.reshape(HEADS, RANK, C)
        fl = W_full.reshape(HEADS, DIM_HEAD, C)
        return np.concatenate([lr, fl], axis=1)

    def slab_lo(Wcat):
        # (H,224,C) -> per group [G, 128(p), CK, HG*128] with layout
        # [g][p, co, hl*128+j] = Wcat[g*HG+hl, j, co*128+p]
        A = Wcat[:, :128, :].reshape(GROUPS, HG, 128, CK, 128)
        return np.ascontiguousarray(A.transpose(0, 4, 3, 1, 2)
                                    .reshape(GROUPS, 128, CK, HG * 128)
                                    ).astype(bf16)

    def slab_hi(Wcat, width, pad_to=None):
        A = Wcat[:, 128:224, :].reshape(GROUPS, HG, 96, CK, 128)
        A = A.transpose(0, 4, 3, 1, 2)  # (G, p, co, hl, 96)
        if pad_to is not None:
            pad = np.zeros(A.shape[:-1] + (pad_to - 96,), A.dtype)
            A = np.concatenate([A, pad], axis=-1)
            width = pad_to
        return np.ascontiguousarray(
            A.reshape(GROUPS, 128, CK, HG * width)).astype(bf16)

    Wq_cat = cat_heads(Wq_lr, Wq_full)
    Wk_cat = cat_heads(Wk_lr, Wk_full)
    Wv_cat = cat_heads(Wv_lr, Wv_full)

    # output projection: Wo_cat (H, 224, C) with Wo_cat[h, d, c] = Wout_cat[c, h*224+d]
    Wo_lr = Wout_lr.reshape(C, HEADS, RANK)
    Wo_fl = Wout_full.reshape(C, HEADS, DIM_HEAD)
    Wo_cat = np.concatenate([Wo_lr, Wo_fl], axis=2).transpose(1, 2, 0)  # (H,224,C)
    wo_lo = np.ascontiguousarray(Wo_cat[:, :128, :]).astype(bf16)
    wo_hi = np.zeros((HEADS, 128, C), np.float32)
    wo_hi[:, :96, :] = Wo_cat[:, 128:, :]
    wo_hi[0, 96, :] = b_out_full  # ones-row of head 0 carries the bias
    wo_hi = wo_hi.astype(bf16)

    ind = np.zeros((128, 8 * 128), np.float32)
    for r in range(8):
        ind[r, r * 128:(r + 1) * 128] = 1.0

    return {
        "wq_lo": slab_lo(Wq_cat), "wq_hi": slab_hi(Wq_cat, 96),
        "wk_lo": slab_lo(Wk_cat), "wk_hi": slab_hi(Wk_cat, 96),
        "wv_lo": slab_lo(Wv_cat), "wv_hi": slab_hi(Wv_cat, 96, pad_to=97),
        "wo_lo": wo_lo, "wo_hi": wo_hi, "ind": ind.astype(bf16),
    }


def _prep_xt(hs_core):
    # (BL, N, C) f32 -> [128, CK, BL*N] bf16, xt[p, co, b*N+n] = x[b, n, co*128+p]
    X = hs_core.reshape(BL * N, CK, 128).transpose(2, 1, 0)
    return np.ascontiguousarray(X).astype(ml_dtypes.bfloat16)


_NC_CACHE = {}


def get_nc():
    if "nc" not in _NC_CACHE:
        _NC_CACHE["nc"] = build_bass()
    return _NC_CACHE["nc"]


def kernel(hidden_states, Wq_lr, Wk_lr, Wv_lr, Wout_lr,
           Wq_full, Wk_full, Wv_full, Wout_full, b_out_full):
    hidden_states = np.asarray(hidden_states, np.float32)
    weights = _prep_weights(
        np.asarray(Wq_lr, np.float32), np.asarray(Wk_lr, np.float32),
        np.asarray(Wv_lr, np.float32), np.asarray(Wout_lr, np.float32),
        np.asarray(Wq_full, np.float32), np.asarray(Wk_full, np.float32),
        np.asarray(Wv_full, np.float32), np.asarray(Wout_full, np.float32),
        np.asarray(b_out_full, np.float32))

    in_maps = []
    for c in range(NCORES):
        m = dict(weights)
        m["xt"] = _prep_xt(hidden_states[c * BL:(c + 1) * BL])
        in_maps.append(m)

    nc = get_nc()
    results = run_bass_kernel_spmd(nc, in_maps, core_ids=list(range(NCORES))).results

    out = np.empty((B, N, C), np.float32)
    for c in range(NCORES):
        o = results[c]["out"]  # (BL, CK, 128, N)
        out[c * BL:(c + 1) * BL] = (
            o.transpose(0, 3, 1, 2).reshape(BL, N, C))
    return out


if __name__ == "__main__":
    nc = get_nc()
    print("built + compiled OK")


# revision 33
# speedup vs baseline: 1.0471x; 1.0158x over previous
"""Trainium2 Bass kernel for dual-branch (low-rank + full-rank) self-attention.

Math (per batch b, head h):
  q = x @ Wq_cat[h].T   (N, 224)   224 = 64 (lr) + 160 (full)
  scoresT[m, n] = sum_d K[m, d] Q[n, d]           (keys m on partitions)
  expT = exp(SCALE * scoresT)                     (no max subtraction; f32 psum)
  xav[d, n] = sum_m Vaug[m, d] expT[m, n]         Vaug has a ones column ->
                                                  row 224 of xav = softmax denom
  xnorm = xav * (1/denom)  (PE K-broadcast of the reciprocal row)
  outT[c, n] = sum_{h,d} Wout_aug[h*225+d, c] * xnorm[h][d, n]
               (ones-row of xnorm x bias-row of Wout_aug adds the bias)

Sharding: data-parallel, 2 batches per core across 8 cores. No collectives.
All matmuls bf16 with f32 PSUM accumulation; softmax + normalization f32.
"""

import os
import sys

sys.path.insert(0, "/opt/trn_rl_repo")

import numpy as np
import ml_dtypes

import concourse.bass as bass
import concourse.mybir as mybir
import concourse.tile as tile
from concourse import bacc
from concourse.bass_utils import run_bass_kernel_spmd

# problem constants (hardcoded per spec)
B, N, C = 16, 1024, 1280
HEADS = 8
RANK = 64
DIM_HEAD = 160
DH = RANK + DIM_HEAD          # 224 concat head dim
SCALE = DIM_HEAD ** (-0.5)
NCORES = 8
BL = B // NCORES              # batches per core = 2
CK = C // 128                 # 10 contraction chunks over C
GROUPS = 2                    # head groups per core pass
HG = HEADS // GROUPS          # 4 heads per group
MC = N // 128                 # 8 key chunks
NT = N // 512                 # 2 query-column tiles

BF16 = mybir.dt.bfloat16
F32 = mybir.dt.float32


def build_bass():
    nc = bacc.Bacc("TRN2", target_bir_lowering=False, debug=False,
                   num_devices=NCORES)

    def din(name, shape, dt=BF16):
        return nc.dram_tensor(name, shape, dt, kind="ExternalInput").ap()

    xt_d = din("xt", [128, CK, BL * N])                 # x transposed, c-major
    wq_lo_d = din("wq_lo", [GROUPS, 128, CK, HG * 128])
    wq_hi_d = din("wq_hi", [GROUPS, 128, CK, HG * 96])
    wk_lo_d = din("wk_lo", [GROUPS, 128, CK, HG * 128])
    wk_hi_d = din("wk_hi", [GROUPS, 128, CK, HG * 96])
    wv_lo_d = din("wv_lo", [GROUPS, 128, CK, HG * 128])
    wv_hi_d = din("wv_hi", [GROUPS, 128, CK, HG * 97])  # 97th col zero (ones col)
    wo_lo_d = din("wo_lo", [HEADS, 128, C])
    wo_hi_d = din("wo_hi", [HEADS, 128, C])             # rows 96..127 zero; row 96 of h=0 = bias
    ind_d = din("ind", [128, 8 * 128])                  # row-broadcast indicator columns
    out_d = nc.dram_tensor("out", [BL, CK, 128, N], F32, kind="ExternalOutput").ap()

    with tile.TileContext(nc) as tc:
        with (
            tc.tile_pool(name="xtp", bufs=1) as xtp,
            tc.tile_pool(name="wp", bufs=3) as wp,
            tc.tile_pool(name="wop", bufs=16) as wop,
            tc.tile_pool(name="qkvp", bufs=1) as qkvp,
            tc.tile_pool(name="xavp", bufs=1) as xavp,
            tc.tile_pool(name="expp", bufs=2) as expp,
            tc.tile_pool(name="cstp", bufs=1) as cstp,
            tc.tile_pool(name="outp", bufs=2) as outp,
            tc.tile_pool(name="xunp", bufs=2) as xunp,
            tc.tile_pool(name="drp", bufs=2, space="DRAM") as drp,
            tc.tile_pool(name="psp", bufs=5, space="PSUM") as psp,
            tc.tile_pool(name="psavp", bufs=3, space="PSUM") as psavp,
        ):
            def ps_tile():
                # general-purpose matmul accumulators (scores/proj/out-proj)
                return psp.tile([128, 512], F32, tag="mm", name="ps")

            def ps_av_tile():
                # AV accumulators + broadcast — separate slots so the
                # normalize chain never blocks the next head's scores
                return psavp.tile([128, 512], F32, tag="av", name="ps_av")

            # constants: indicator columns for row-broadcast matmuls
            # ind[:, r*128:(r+1)*128] has ones in row r only, so
            # matmul(out, ind[:, r-slice], recip_all) broadcasts row r

            pending_out = []
            pending_norm = []

            def emit_out_proj():
                # output projection (c on partitions; host untransposes)
                ob, oxav_lo, oxav_hi = pending_out.pop(0)
                wo_lo_t = []
                wo_hi_t = []
                for h in range(HEADS):
                    wl = wop.tile([128, C], BF16, tag="wo", name="wo")
                    nc.sync.dma_start(wl[:], wo_lo_d[h])
                    wh = wop.tile([128, C], BF16, tag="wo", name="wo")
                    nc.sync.dma_start(wh[:], wo_hi_d[h])
                    wo_lo_t.append(wl)
                    wo_hi_t.append(wh)
                for ct in range(CK):
                    for nt in range(NT):
                        ps_o = ps_tile()
                        for h in range(HEADS):
                            nc.tensor.matmul(
                                ps_o[:],
                                wo_lo_t[h][:, ct * 128:(ct + 1) * 128],
                                oxav_lo[:, h, nt * 512:(nt + 1) * 512],
                                start=(h == 0), stop=False)
                            nc.tensor.matmul(
                                ps_o[:],
                                wo_hi_t[h][:, ct * 128:(ct + 1) * 128],
                                oxav_hi[:, h, nt * 512:(nt + 1) * 512],
                                start=False, stop=(h == HEADS - 1))
                        ot = outp.tile([128, 512], F32, tag="ot", name="ot")
                        nc.vector.tensor_copy(ot[:], ps_o[:])
                        nc.sync.dma_start(
                            out_d[ob, ct, :, nt * 512:(nt + 1) * 512], ot[:])

            for b in range(BL):
                xt = xtp.tile([128, CK, N], BF16, tag="xt")
                for co in range(CK):  # per-chunk loads: compute starts sooner
                    nc.sync.dma_start(xt[:, co, :],
                                      xt_d[:, co, b * N:(b + 1) * N])

                xav_lo = xavp.tile([128, HEADS, N], BF16, tag="xav_lo")
                xav_hi = xavp.tile([128, HEADS, N], BF16, tag="xav_hi")
                nc.gpsimd.memset(xav_hi[96:128, :, :], 0.0)

                for g in range(GROUPS):
                    # ---- stream this group's projection weights ----
                    def load_slab(dram, width):
                        t = wp.tile([128, CK, width], BF16, tag="wslab",
                                    name="wslab")
                        for co in range(CK):  # per-chunk: spread across queues
                            nc.sync.dma_start(t[:, co, :], dram[g, :, co, :])
                        return t
                    w_qlo = load_slab(wq_lo_d, HG * 128)
                    w_qhi = load_slab(wq_hi_d, HG * 96)
                    w_klo = load_slab(wk_lo_d, HG * 128)
                    w_khi = load_slab(wk_hi_d, HG * 96)
                    w_vlo = load_slab(wv_lo_d, HG * 128)
                    w_vhi = load_slab(wv_hi_d, HG * 97)

                    # ---- projections: qT/kT (head-dim on partitions) ----
                    qt_lo = qkvp.tile([128, HG, N], BF16, tag="qt_lo")
                    qt_hi = qkvp.tile([128, HG, N], BF16, tag="qt_hi")
                    kt_lo = qkvp.tile([128, HG, N], BF16, tag="kt_lo")
                    kt_hi = qkvp.tile([128, HG, N], BF16, tag="kt_hi")
                    nc.gpsimd.memset(qt_hi[96:128, :, :], 0.0)
                    nc.gpsimd.memset(kt_hi[96:128, :, :], 0.0)

                    # hi chunks of 4 heads (4x96=384 rows) pack into 3 full
                    # M=128 matmuls; fragments unmixed by the psum->sbuf
                    # copies (pieces respect the 32-aligned partition-window
                    # rule: base 0 may span freely, other bases max 32 rows)
                    HI_FRAGS = {  # ch -> [(hl, src_base, dst_base, rows)]
                        0: [(0, 0, 0, 96), (1, 96, 0, 32)],
                        1: [(1, 0, 32, 32), (1, 32, 64, 32),
                            (2, 64, 0, 32), (2, 96, 32, 32)],
                        2: [(2, 0, 64, 32), (3, 32, 0, 32),
                            (3, 64, 32, 32), (3, 96, 64, 32)],
                    }
                    for (wlo, whi, tlo, thi) in (
                        (w_qlo, w_qhi, qt_lo, qt_hi),
                        (w_klo, w_khi, kt_lo, kt_hi),
                    ):
                        for hl in range(HG):
                            for nt in range(NT):
                                ps = ps_tile()
                                for co in range(CK):
                                    nc.tensor.matmul(
                                        ps[:],
                                        wlo[:, co, hl * 128:(hl + 1) * 128],
                                        xt[:, co, nt * 512:(nt + 1) * 512],
                                        start=(co == 0), stop=(co == CK - 1))
                                nc.vector.tensor_copy(
                                    tlo[:, hl, nt * 512:(nt + 1) * 512], ps[:])
                        for ch in range(3):
                            for nt in range(NT):
                                ps = ps_tile()
                                for co in range(CK):
                                    nc.tensor.matmul(
                                        ps[:],
                                        whi[:, co, ch * 128:(ch + 1) * 128],
                                        xt[:, co, nt * 512:(nt + 1) * 512],
                                        start=(co == 0), stop=(co == CK - 1))
                                for (hl, sb, db, rows) in HI_FRAGS[ch]:
                                    nc.vector.tensor_copy(
                                        thi[db:db + rows, hl,
                                            nt * 512:(nt + 1) * 512],
                                        ps[sb:sb + rows, :])

                    # ---- V projection: natural layout (keys on partitions) ----
                    v_lo = qkvp.tile([128, MC, HG * 128], BF16, tag="v_lo")
                    v_hi = qkvp.tile([128, MC, HG * 97], BF16, tag="v_hi")
                    for mc in range(MC):
                        ps_l = ps_tile()
                        ps_h = ps_tile()
                        for co in range(CK):
                            nc.tensor.matmul(
                                ps_l[:],
                                xt[:, co, mc * 128:(mc + 1) * 128],
                                w_vlo[:, co, :],
                                start=(co == 0), stop=(co == CK - 1))
                        for co in range(CK):
                            nc.tensor.matmul(
                                ps_h[:, 0:HG * 97],
                                xt[:, co, mc * 128:(mc + 1) * 128],
                                w_vhi[:, co, :],
                                start=(co == 0), stop=(co == CK - 1))
                        nc.vector.tensor_copy(v_lo[:, mc, :], ps_l[:])
                        nc.vector.tensor_copy(v_hi[:, mc, :], ps_h[:, 0:HG * 97])
                    # ones column per head (softmax denominator row source)
                    for hl in range(HG):
                        nc.gpsimd.memset(v_hi[:, :, hl * 97 + 96], 1.0)

                    # emit deferred work here (after this group's
                    # projections): previous group's normalize chain, then
                    # the previous batch's output projection, so the PE chews
                    # dense matmuls while those chains settle
                    if pending_norm:
                        pending_norm.pop(0)()
                    if g == 0 and pending_out:
                        emit_out_proj()

                    # ---- attention per head: unnormalized AV into xav ----
                    for hl in range(HG):
                        h_abs = g * HG + hl
                        for nt in range(NT):
                            expt = expp.tile([128, MC * 512], BF16, tag="expt")
                            for mc in range(MC):
                                ps_sc = ps_tile()
                                nc.tensor.matmul(
                                    ps_sc[:],
                                    kt_lo[:, hl, mc * 128:(mc + 1) * 128],
                                    qt_lo[:, hl, nt * 512:(nt + 1) * 512],
                                    start=True, stop=False)
                                nc.tensor.matmul(
                                    ps_sc[:],
                                    kt_hi[:, hl, mc * 128:(mc + 1) * 128],
                                    qt_hi[:, hl, nt * 512:(nt + 1) * 512],
                                    start=False, stop=True)
                                nc.scalar.activation(
                                    expt[:, mc * 512:(mc + 1) * 512],
                                    ps_sc[:],
                                    mybir.ActivationFunctionType.Exp,
                                    scale=SCALE)
                            ps_alo = ps_av_tile()
                            ps_ahi = ps_av_tile()
                            for mc in range(MC):
                                nc.tensor.matmul(
                                    ps_alo[:],
                                    v_lo[:, mc, hl * 128:(hl + 1) * 128],
                                    expt[:, mc * 512:(mc + 1) * 512],
                                    start=(mc == 0), stop=(mc == MC - 1))
                            for mc in range(MC):
                                nc.tensor.matmul(
                                    ps_ahi[0:97, :],
                                    v_hi[:, mc, hl * 97:(hl + 1) * 97],
                                    expt[:, mc * 512:(mc + 1) * 512],
                                    start=(mc == 0), stop=(mc == MC - 1))
                            # drain PSUM immediately (unnormalized)
                            nc.vector.tensor_copy(
                                xav_lo[:, h_abs, nt * 512:(nt + 1) * 512],
                                ps_alo[:])
                            nc.vector.tensor_copy(
                                xav_hi[0:97, h_abs, nt * 512:(nt + 1) * 512],
                                ps_ahi[0:97, :])

                    # gather this group's 8 denominator rows onto 8 partitions
                    # (DMA via DRAM bounce: compute engines can't write
                    # partition offsets 1..7, and SBUF APs can't move free
                    # elements across partitions)
                    s_scr = drp.tile([1, HG, N], BF16, tag="s_scr")
                    nc.sync.dma_start(
                        s_scr[:], xav_hi[96:97, g * HG:(g + 1) * HG, :])
                    s_all = xunp.tile([8, 512], BF16, tag="s_all")
                    nc.sync.dma_start(
                        s_all[:],
                        s_scr[:].rearrange("o h (t i) -> (o h t) i", t=NT))

                    # ---- batched softmax denominators: one 8-lane recip,
                    # then partition-broadcast via stride-0 DRAM-read DMAs.
                    # Deferred: emitted after the NEXT phase's projections so
                    # the 4us recip + 16 DVE mults don't stall the DVE queue
                    # ahead of the projection psum->sbuf copies.
                    def norm_closure(g=g, s_all=s_all, xav_lo=xav_lo,
                                     xav_hi=xav_hi):
                        recip_all = xunp.tile([8, 512], BF16, tag="recip_all",
                                              name="recip_all")
                        with nc.allow_low_precision(reason="denom recip bf16"):
                            nc.vector.reciprocal(recip_all[:], s_all[:])
                        r_scr = drp.tile([8, 512], BF16, tag="r_scr",
                                         name="r_scr")
                        nc.sync.dma_start(r_scr[:], recip_all[:])
                        for hl in range(HG):
                            h_abs = g * HG + hl
                            for nt in range(NT):
                                sel = hl * NT + nt
                                bc_sb = outp.tile([128, 512], BF16,
                                                  tag="bc_sb", name="bc_sb")
                                nc.sync.dma_start(
                                    bc_sb[:],
                                    r_scr[sel:sel + 1, :]
                                    .to_broadcast([128, 512]))
                                nc.vector.tensor_tensor(
                                    xav_lo[:, h_abs, nt * 512:(nt + 1) * 512],
                                    xav_lo[:, h_abs, nt * 512:(nt + 1) * 512],
                                    bc_sb[:], mybir.AluOpType.mult)
                                nc.vector.tensor_tensor(
                                    xav_hi[0:97, h_abs,
                                           nt * 512:(nt + 1) * 512],
                                    xav_hi[0:97, h_abs,
                                           nt * 512:(nt + 1) * 512],
                                    bc_sb[0:97, :], mybir.AluOpType.mult)
                    pending_norm.append(norm_closure)

                pending_out.append((b, xav_lo, xav_hi))

            while pending_norm:
                pending_norm.pop(0)()
            while pending_out:
                emit_out_proj()

    nc.compile()
    return nc


def _prep_weights(Wq_lr, Wk_lr, Wv_lr, Wout_lr, Wq_full, Wk_full, Wv_full,
                  Wout_full, b_out_full):
    """Host-side weight concat/transpose into device layouts (bf16)."""
    bf16 = ml_dtypes.bfloat16

    def cat_heads(W_lr, W_full):
        # -> (H, 224, C)
        lr = W_lr.reshape(HEADS, RANK, C)
        fl = W_full.reshape(HEADS, DIM_HEAD, C)
        return np.concatenate([lr, fl], axis=1)

    def slab_lo(Wcat):
        # (H,224,C) -> per group [G, 128(p), CK, HG*128] with layout
        # [g][p, co, hl*128+j] = Wcat[g*HG+hl, j, co*128+p]
        A = Wcat[:, :128, :].reshape(GROUPS, HG, 128, CK, 128)
        return np.ascontiguousarray(A.transpose(0, 4, 3, 1, 2)
                                    .reshape(GROUPS, 128, CK, HG * 128)
                                    ).astype(bf16)

    def slab_hi(Wcat, width, pad_to=None):
        A = Wcat[:, 128:224, :].reshape(GROUPS, HG, 96, CK, 128)
        A = A.transpose(0, 4, 3, 1, 2)  # (G, p, co, hl, 96)
        if pad_to is not None:
            pad = np.zeros(A.shape[:-1] + (pad_to - 96,), A.dtype)
            A = np.concatenate([A, pad], axis=-1)
            width = pad_to
        return np.ascontiguousarray(
            A.reshape(GROUPS, 128, CK, HG * width)).astype(bf16)

    Wq_cat = cat_heads(Wq_lr, Wq_full)
    Wk_cat = cat_heads(Wk_lr, Wk_full)
    Wv_cat = cat_heads(Wv_lr, Wv_full)

    # output projection: Wo_cat (H, 224, C) with Wo_cat[h, d, c] = Wout_cat[c, h*224+d]
    Wo_lr = Wout_lr.reshape(C, HEADS, RANK)
    Wo_fl = Wout_full.reshape(C, HEADS, DIM_HEAD)
    Wo_cat = np.concatenate([Wo_lr, Wo_fl], axis=2).transpose(1, 2, 0)  # (H,224,C)
    wo_lo = np.ascontiguousarray(Wo_cat[:, :128, :]).astype(bf16)
    wo_hi = np.zeros((HEADS, 128, C), np.float32)
    wo_hi[:, :96, :] = Wo_cat[:, 128:, :]
    wo_hi[0, 96, :] = b_out_full  # ones-row of head 0 carries the bias
    wo_hi = wo_hi.astype(bf16)

    ind = np.zeros((128, 8 * 128), np.float32)
    for r in range(8):
        ind[r, r * 128:(r + 1) * 128] = 1.0

    return {
        "wq_lo": slab_lo(Wq_cat), "wq_hi": slab_hi(Wq_cat, 96),
        "wk_lo": slab_lo(Wk_cat), "wk_hi": slab_hi(Wk_cat, 96),
        "wv_lo": slab_lo(Wv_cat), "wv_hi": slab_hi(Wv_cat, 96, pad_to=97),
        "wo_lo": wo_lo, "wo_hi": wo_hi, "ind": ind.astype(bf16),
    }


def _prep_xt(hs_core):
    # (BL, N, C) f32 -> [128, CK, BL*N] bf16, xt[p, co, b*N+n] = x[b, n, co*128+p]
    X = hs_core.reshape(BL * N, CK, 128).transpose(2, 1, 0)
    return np.ascontiguousarray(X).astype(ml_dtypes.bfloat16)


_NC_CACHE = {}


def get_nc():
    if "nc" not in _NC_CACHE:
        _NC_CACHE["nc"] = build_bass()
    return _NC_CACHE["nc"]


def kernel(hidden_states, Wq_lr, Wk_lr, Wv_lr, Wout_lr,
           Wq_full, Wk_full, Wv_full, Wout_full, b_out_full):
    hidden_states = np.asarray(hidden_states, np.float32)
    weights = _prep_weights(
        np.asarray(Wq_lr, np.float32), np.asarray(Wk_lr, np.float32),
        np.asarray(Wv_lr, np.float32), np.asarray(Wout_lr, np.float32),
        np.asarray(Wq_full, np.float32), np.asarray(Wk_full, np.float32),
        np.asarray(Wv_full, np.float32), np.asarray(Wout_full, np.float32),
        np.asarray(b_out_full, np.float32))

    in_maps = []
    for c in range(NCORES):
        m = dict(weights)
        m["xt"] = _prep_xt(hidden_states[c * BL:(c + 1) * BL])
        in_maps.append(m)

    nc = get_nc()
    results = run_bass_kernel_spmd(nc, in_maps, core_ids=list(range(NCORES))).results

    out = np.empty((B, N, C), np.float32)
    for c in range(NCORES):
        o = results[c]["out"]  # (BL, CK, 128, N)
        out[c * BL:(c + 1) * BL] = (
            o.transpose(0, 3, 1, 2).reshape(BL, N, C))
    return out


if __name__ == "__main__":
    nc = get_nc()
    print("built + compiled OK")
